# revision 17
# baseline (speedup 1.0000x reference)
"""Trainium2 Bass kernel for nn_Attention_86638080295542.

Multi-head attention (12 heads, d=64) with the reference's v=k quirk:
    q = x @ Wq.T + bq ; k = x @ Wk.T + bk ; v = k
    out = softmax(q k^T / sqrt(d)) @ v ;  y = out @ Wo.T + bo

Sharding: batch (B=8) data-parallel across the 8 NeuronCores — core c
computes batch element c end-to-end, no collectives.

Per-core dataflow (all "T" tensors keep the contraction dim on SBUF
partitions so every matmul is a natural lhsT.T @ rhs):
  xT[e,s], WqT/WkT/WoT[e_in,e_out] are pre-transposed on the host.
  qT = Wq @ xT   (+bq per-partition)        [768,1024]
  kT = Wk @ xT   (+bk per-partition)        [768,1024]
  vaug[j, jb, h, 0:64] = k natural (PE transpose of kT), col 64 = 1.0
  per head h: pT[j,i] = exp(scale * kT_h^T qT_h)  (no max-subtraction:
     logits are O(1) for this problem, softmax is shift-invariant)
  outT_h[d,i] (+ rowsum in row 64) = vaug^T @ pT, accumulated over j
  normalize: outT_h *= 1/rowsum (broadcast via ones-matmul on PE)
  y = outT^T @ WoT + bo
"""

from contextlib import ExitStack

import numpy as np

import concourse.bass as bass
import concourse.tile as tile
from concourse import bacc, mybir
from concourse import bass_utils

S = 1024          # sequence length
E = 768           # embed dim
H = 12            # heads
DH = 64           # head dim
P = 128           # partitions
KT = E // P       # 6 k-tiles over embed dim
ST = S // P       # 8 tiles over sequence
NCH = S // 512    # 2 free-dim chunks of 512 over sequence
SCALE = DH ** -0.5
NCORES = 8

F32 = mybir.dt.float32
F32R = mybir.dt.float32r
BF16 = mybir.dt.bfloat16


def _emit(nc, tc, ctx, iters=1, variant='full'):
    xT_d = nc.dram_tensor("xT", [E, S], BF16, kind="ExternalInput")
    WqT_d = nc.dram_tensor("WqT", [E, E], BF16, kind="ExternalInput")
    WkT_d = nc.dram_tensor("WkT", [E, E], BF16, kind="ExternalInput")
    WoT_d = nc.dram_tensor("WoT", [E, E], BF16, kind="ExternalInput")
    bq_d = nc.dram_tensor("bq", [E], F32, kind="ExternalInput")
    bk_d = nc.dram_tensor("bk", [E], F32, kind="ExternalInput")
    bo_d = nc.dram_tensor("bo", [E], F32, kind="ExternalInput")
    y_d = nc.dram_tensor("y", [S, E], F32, kind="ExternalOutput")

    Exp = mybir.ActivationFunctionType.Exp

    const = ctx.enter_context(tc.tile_pool(name="const", bufs=1))
    xt_pool = ctx.enter_context(tc.tile_pool(name="xt", bufs=1))
    outt_pool = ctx.enter_context(tc.tile_pool(name="outt", bufs=1))
    w_pool = ctx.enter_context(tc.tile_pool(name="w", bufs=2))
    wo_pool = ctx.enter_context(tc.tile_pool(name="wo", bufs=1))
    vaug_pool = ctx.enter_context(tc.tile_pool(name="vaug", bufs=1))
    qt_pool = ctx.enter_context(tc.tile_pool(name="qt", bufs=3))
    kt_pool = ctx.enter_context(tc.tile_pool(name="kt", bufs=3))
    pt_pool = ctx.enter_context(tc.tile_pool(name="pt", bufs=8))
    ysb_pool = ctx.enter_context(tc.tile_pool(name="ysb", bufs=8))
    pvsb_pool = ctx.enter_context(tc.tile_pool(name="pvsb", bufs=4))
    rc_pool = ctx.enter_context(tc.tile_pool(name="rc", bufs=4))
    rb_pool = ctx.enter_context(tc.tile_pool(name="rb", bufs=2))
    # PSUM budget (8 banks): scores 2x[128,1024]f32 = 4, pv 1x[65,1024]f32 = 2,
    # proj/transpose/outproj 2x[128,512] = 2.
    ps_s = ctx.enter_context(tc.tile_pool(name="ps_s", bufs=2, space="PSUM"))
    ps_pv = ctx.enter_context(tc.tile_pool(name="ps_pv", bufs=1, space="PSUM"))
    ps_w = ctx.enter_context(tc.tile_pool(name="ps_w", bufs=2, space="PSUM"))
    dram_pool = ctx.enter_context(tc.tile_pool(name="dram", bufs=4, space="DRAM"))

    # ---- constants ----
    # gpsimd/memset can't emit float32r, so build fp32 then copy-round on DVE
    # (0.0/1.0 are exactly representable, so the copy is exact).
    ident_f32 = const.tile([P, P], F32, tag="ident_f32")
    from concourse.masks import make_identity
    make_identity(nc, ident_f32[:])
    identity = const.tile([P, P], BF16, tag="ident")
    nc.vector.tensor_copy(identity[:], ident_f32[:])
    bq_sb = const.tile([P, KT], F32, tag="bq")
    nc.sync.dma_start(bq_sb[:], bq_d.ap().rearrange("(t p) -> p t", p=P))
    bk_sb = const.tile([P, KT], F32, tag="bk")
    nc.sync.dma_start(bk_sb[:], bk_d.ap().rearrange("(t p) -> p t", p=P))
    # bo broadcast to all 128 partitions via a 0-step partition AP (DRAM APs
    # are not partitioned, so a 0-step leading dim is legal here)
    bo_bc = const.tile([P, E], F32, tag="bo")
    bo_ap = bo_d.ap()
    bo_bcast_src = bass.AP(bo_ap.tensor, bo_ap.offset, [[0, P], [1, E]])
    nc.sync.dma_start(bo_bc[:], bo_bcast_src)

    # ---- input loads (per k-tile so compute can start early) ----
    xT_sb = xt_pool.tile([P, KT, S], BF16, tag="xt")
    WqT_sb = w_pool.tile([P, KT, E], BF16, tag="w")
    WkT_sb = w_pool.tile([P, KT, E], BF16, tag="w")
    WoT_sb = wo_pool.tile([P, KT, E], BF16, tag="wo")
    xT_r = xT_d.ap().rearrange("(t p) s -> p t s", p=P)
    WqT_r = WqT_d.ap().rearrange("(t p) e -> p t e", p=P)
    WkT_r = WkT_d.ap().rearrange("(t p) e -> p t e", p=P)
    WoT_r = WoT_d.ap().rearrange("(t p) e -> p t e", p=P)
    for t in range(KT):
        nc.sync.dma_start(xT_sb[:, t, :], xT_r[:, t, :])
        nc.sync.dma_start(WqT_sb[:, t, :], WqT_r[:, t, :])
        nc.sync.dma_start(WkT_sb[:, t, :], WkT_r[:, t, :])
        nc.sync.dma_start(WoT_sb[:, t, :], WoT_r[:, t, :])

    vaug = vaug_pool.tile([P, ST, H, DH + 1], BF16, tag="vaug")
    for jb in range(ST):
        nc.vector.memset(vaug[:, jb, :, DH:DH + 1], 1.0)
    outT_sb = outt_pool.tile([P, KT, S], BF16, tag="outt")

    # warm the exp activation-table set once, outside the hardware loop, so
    # ACT_TABLE_LOAD (~1.3us) isn't paid per iteration
    wsrc = const.tile([1, 4], F32, tag="wsrc")
    nc.vector.memset(wsrc[:], 0.0)
    wdst = const.tile([1, 4], F32, tag="wdst")
    nc.scalar.activation(wdst[:], wsrc[:], Exp, scale=1.0)

    if iters > 1:
        ctx.enter_context(tc.For_i(0, iters, 1))

    # ---- pair preparation: projections + vaug transposes for head pair hp,
    # written as a generator that yields after every couple of PE ops. The
    # attention loop of pair hp-1 drains it a few steps per j-block, so this
    # PE work fills the idle slivers while ACT runs exp — keeping the tensor
    # engine dense enough that the HAM clock gate stays at full speed.
    pair_qk = {}

    def pair_prep(hp):
        qp = qt_pool.tile([P, S], BF16, tag="qt", name=f"qp_{hp}")
        kp = kt_pool.tile([P, S], BF16, tag="kt", name=f"kp_{hp}")
        pair_qk[hp] = (qp, kp)
        for W_sb, b_sb, out_sb in ((WqT_sb, bq_sb, qp), (WkT_sb, bk_sb, kp)):
            for c in range(NCH):
                ps = ps_w.tile([P, 512], F32, tag="ps_w")
                for t in range(KT):
                    nc.tensor.matmul(
                        ps[:],
                        W_sb[:, t, 128 * hp:128 * hp + 128],
                        xT_sb[:, t, 512 * c:512 * c + 512],
                        start=(t == 0), stop=(t == KT - 1),
                    )
                    if t % 2 == 1:
                        yield
                nc.vector.tensor_scalar_add(
                    out_sb[:, 512 * c:512 * c + 512], ps[:], b_sb[:, hp:hp + 1]
                )
                yield
        # vaug slices for heads 2hp, 2hp+1 via PE transposes of kT tile hp
        for g in range(2):
            ps = ps_w.tile([P, 512], BF16, tag="ps_w")
            for j4 in range(4):
                jb = 4 * g + j4
                nc.tensor.transpose(
                    ps[:, 128 * j4:128 * j4 + 128],
                    kp[:, 128 * jb:128 * jb + 128],
                    identity[:],
                )
                if j4 % 2 == 1:
                    yield
            nc.vector.tensor_copy(
                vaug[:, 4 * g:4 * g + 4, 2 * hp:2 * hp + 2, 0:DH],
                ps[:].rearrange("p (a b c) -> p a b c", a=4, b=2, c=DH),
            )
            yield

    def drain(gen, n):
        if gen is None:
            return None
        for _ in range(n):
            try:
                next(gen)
            except StopIteration:
                return None
        return gen

    # partial output projection: the t=0..3 outT tiles only depend on head
    # pairs 0..3, so their contributions to y accumulate during pair 5's
    # attention (as its interleave filler) and evict early into ysb partials;
    # only the t=4,5 matmuls remain for the epilogue after the last head.
    ysb_tiles = {}

    def outproj_prep():
        for st in range(ST):
            ysb = ysb_pool.tile([P, E], F32, tag="ysb", name=f"ysb_{st}")
            ysb_tiles[st] = ysb
            for n0 in (0, 384):
                ps = ps_w.tile([P, 512], F32, tag="ps_w")
                for t in range(KT - 2):
                    nc.tensor.matmul(
                        ps[:, 0:384],
                        outT_sb[:, t, 128 * st:128 * st + 128],
                        WoT_sb[:, t, n0:n0 + 384],
                        start=(t == 0), stop=(t == KT - 3),
                    )
                    if t % 2 == 1:
                        yield
                nc.vector.tensor_add(
                    ysb[:, n0:n0 + 384], ps[:, 0:384], bo_bc[:, n0:n0 + 384])
                yield

    # prologue: pair 0 prepared densely before its attention starts
    for _ in pair_prep(0):
        pass

    for hp in range(KT):
        qp, kp = pair_qk.pop(hp)
        last_pair = hp + 1 >= KT
        gen = outproj_prep() if last_pair else pair_prep(hp + 1)
        nsteps = 3 if last_pair else 2
        for h in (2 * hp, 2 * hp + 1):
            po = DH * (h % 2)
            pv = ps_pv.tile([DH + 1, S], F32, tag="ps_pv", name=f"pv_{h}")

            def pv_mms(jb, pt):
                for c in range(NCH):
                    nc.tensor.matmul(
                        pv[:, 512 * c:512 * c + 512],
                        vaug[:, jb, h, :],
                        pt[:, 512 * c:512 * c + 512],
                        start=(jb == 0), stop=(jb == ST - 1),
                    )

            # software-pipelined by one j-block: the PE issues scores(jb)
            # before PV(jb-1), so exp(jb-1) on ACT overlaps scores(jb) on PE
            # instead of stalling the PE.
            prev = None
            for jb in range(ST):
                sps = ps_s.tile([P, S], F32, tag="ps_s", name=f"sps_{h}_{jb}")
                for c in range(NCH):
                    nc.tensor.matmul(
                        sps[:, 512 * c:512 * c + 512],
                        kp[po:po + DH, 128 * jb:128 * jb + 128],
                        qp[po:po + DH, 512 * c:512 * c + 512],
                        start=True, stop=True,
                    )
                pt = pt_pool.tile([P, S], BF16, tag="pt")
                nc.scalar.activation(pt[:], sps[:], Exp, scale=SCALE)
                gen = drain(gen, nsteps)
                if prev is not None:
                    pv_mms(jb - 1, prev)
                prev = pt
            pv_mms(ST - 1, prev)
            # evict pv to SBUF right away (frees the PSUM bank), then
            # normalize: reciprocal of the rowsum row, broadcast across 64
            # partitions via a DRAM round-trip (DRAM APs allow a 0-step
            # partition dim), multiply into outT. Keeps the PE entirely out
            # of the normalization chain.
            pvsb = pvsb_pool.tile([DH + 1, S], F32, tag="pvsb", name=f"pvsb_{h}")
            nc.vector.tensor_copy(pvsb[:], pv[:])
            # reciprocal of the rowsum (pv row DH): a [1, S] DVE reciprocal
            # is ~6.4ns/elem on a single lane (6.5us), so spread it across a
            # [128, S/128] layout with an SBUF->SBUF DMA (dst walks p-major:
            # rs8[p, e] = rowsum[8p + e]), reciprocal on all 128 lanes
            # (~0.2us), scatter back to a partition-0 [1, S] tile, and
            # broadcast across 64 partitions on the idle GPSIMD engine.
            # (DVE operand base partitions must stay 32-aligned: inputs to
            # the mul are all at base 0.)
            rs8 = rc_pool.tile([P, S // P], F32, tag="rs8", name=f"rs8_{h}")
            nc.sync.dma_start(rs8[:], pvsb[DH:DH + 1, :])
            rc8 = rc_pool.tile([P, S // P], F32, tag="rc8", name=f"rc8_{h}")
            nc.vector.reciprocal(rc8[:], rs8[:])
            rcl = rc_pool.tile([1, S], F32, tag="rcl", name=f"rcl_{h}")
            nc.sync.dma_start(rcl[:], rc8[:])
            rb = rb_pool.tile([DH, S], F32, tag="rb", name=f"rb_{h}")
            nc.gpsimd.partition_broadcast(rb[:], rcl[:])
            nc.vector.tensor_mul(
                outT_sb[po:po + DH, hp, :], pvsb[0:DH, :], rb[:],
            )

        drain(gen, 999)

    # ---- output projection epilogue: the last two outT tiles (pairs 4, 5)
    # into each ysb partial, then store y ----
    y_r = y_d.ap().rearrange("(st p) e -> st p e", p=P)
    for st in range(ST):
        ysb = ysb_tiles.pop(st)
        for n0 in (0, 384):
            yps = ps_w.tile([P, 512], F32, tag="ps_w")
            for t in (KT - 2, KT - 1):
                nc.tensor.matmul(
                    yps[:, 0:384],
                    outT_sb[:, t, 128 * st:128 * st + 128],
                    WoT_sb[:, t, n0:n0 + 384],
                    start=(t == KT - 2), stop=(t == KT - 1),
                )
            nc.vector.tensor_add(ysb[:, n0:n0 + 384], ysb[:, n0:n0 + 384], yps[:, 0:384])
        nc.sync.dma_start(y_r[st], ysb[:])


_NC_CACHE = {}


def build(iters=1, variant="full"):
    key = (iters, variant)
    nc = _NC_CACHE.get(key)
    if nc is None:
        nc = bacc.Bacc("TRN2", target_bir_lowering=False, debug=False)
        with tile.TileContext(nc) as tc, ExitStack() as ctx:
            _emit(nc, tc, ctx, iters=iters, variant=variant)
        nc.compile()
        _NC_CACHE[key] = nc
    return nc


def _to_bf16(a):
    return np.ascontiguousarray(
        np.asarray(a, dtype=np.float32)).astype(mybir.dt.np(mybir.dt.bfloat16))


def make_in_maps(x, Wq, bq, Wk, bk, Wo, bo):
    WqT = _to_bf16(np.asarray(Wq, dtype=np.float32).T)
    WkT = _to_bf16(np.asarray(Wk, dtype=np.float32).T)
    WoT = _to_bf16(np.asarray(Wo, dtype=np.float32).T)
    bq = np.ascontiguousarray(np.asarray(bq, dtype=np.float32))
    bk = np.ascontiguousarray(np.asarray(bk, dtype=np.float32))
    bo = np.ascontiguousarray(np.asarray(bo, dtype=np.float32))
    x = np.asarray(x, dtype=np.float32)
    return [
        {
            "xT": _to_bf16(x[c].T),
            "WqT": WqT, "WkT": WkT, "WoT": WoT,
            "bq": bq, "bk": bk, "bo": bo,
        }
        for c in range(NCORES)
    ]


def kernel(x, Wq, bq, Wk, bk, Wo, bo):
    nc = build()
    in_maps = make_in_maps(x, Wq, bq, Wk, bk, Wo, bo)
    res = bass_utils.run_bass_kernel_spmd(nc, in_maps, core_ids=list(range(NCORES)))
    return np.stack([res.results[c]["y"] for c in range(NCORES)]).astype(np.float32)



# revision 19
# speedup vs baseline: 496.3072x; 496.3072x over previous
"""Trainium2 Bass kernel for nn_Attention_86638080295542.

Multi-head attention (12 heads, d=64) with the reference's v=k quirk:
    q = x @ Wq.T + bq ; k = x @ Wk.T + bk ; v = k
    out = softmax(q k^T / sqrt(d)) @ v ;  y = out @ Wo.T + bo

Sharding: batch (B=8) data-parallel across the 8 NeuronCores — core c
computes batch element c end-to-end, no collectives.

Per-core dataflow (all "T" tensors keep the contraction dim on SBUF
partitions so every matmul is a natural lhsT.T @ rhs):
  xT[e,s], WqT/WkT/WoT[e_in,e_out] are pre-transposed on the host.
  qT = Wq @ xT   (+bq per-partition)        [768,1024]
  kT = Wk @ xT   (+bk per-partition)        [768,1024]
  vaug[j, jb, h, 0:64] = k natural (PE transpose of kT), col 64 = 1.0
  per head h: pT[j,i] = exp(scale * kT_h^T qT_h)  (no max-subtraction:
     logits are O(1) for this problem, softmax is shift-invariant)
  outT_h[d,i] (+ rowsum in row 64) = vaug^T @ pT, accumulated over j
  normalize: outT_h *= 1/rowsum (broadcast via ones-matmul on PE)
  y = outT^T @ WoT + bo
"""

from contextlib import ExitStack

import numpy as np

import concourse.bass as bass
import concourse.tile as tile
from concourse import bacc, mybir
from concourse import bass_utils

S = 1024          # sequence length
E = 768           # embed dim
H = 12            # heads
DH = 64           # head dim
P = 128           # partitions
KT = E // P       # 6 k-tiles over embed dim
ST = S // P       # 8 tiles over sequence
NCH = S // 512    # 2 free-dim chunks of 512 over sequence
SCALE = DH ** -0.5
NCORES = 8

F32 = mybir.dt.float32
F32R = mybir.dt.float32r
BF16 = mybir.dt.bfloat16


def _emit(nc, tc, ctx, iters=1, variant='full'):
    xT_d = nc.dram_tensor("xT", [E, S], BF16, kind="ExternalInput")
    WqT_d = nc.dram_tensor("WqT", [E, E], BF16, kind="ExternalInput")
    WkT_d = nc.dram_tensor("WkT", [E, E], BF16, kind="ExternalInput")
    WoT_d = nc.dram_tensor("WoT", [E, E], BF16, kind="ExternalInput")
    bq_d = nc.dram_tensor("bq", [E], F32, kind="ExternalInput")
    bk_d = nc.dram_tensor("bk", [E], F32, kind="ExternalInput")
    bo_d = nc.dram_tensor("bo", [E], F32, kind="ExternalInput")
    y_d = nc.dram_tensor("y", [S, E], F32, kind="ExternalOutput")

    Exp = mybir.ActivationFunctionType.Exp

    const = ctx.enter_context(tc.tile_pool(name="const", bufs=1))
    xt_pool = ctx.enter_context(tc.tile_pool(name="xt", bufs=1))
    outt_pool = ctx.enter_context(tc.tile_pool(name="outt", bufs=1))
    w_pool = ctx.enter_context(tc.tile_pool(name="w", bufs=2))
    wo_pool = ctx.enter_context(tc.tile_pool(name="wo", bufs=1))
    vaug_pool = ctx.enter_context(tc.tile_pool(name="vaug", bufs=1))
    qt_pool = ctx.enter_context(tc.tile_pool(name="qt", bufs=3))
    kt_pool = ctx.enter_context(tc.tile_pool(name="kt", bufs=3))
    pt_pool = ctx.enter_context(tc.tile_pool(name="pt", bufs=8))
    ysb_pool = ctx.enter_context(tc.tile_pool(name="ysb", bufs=8))
    pvsb_pool = ctx.enter_context(tc.tile_pool(name="pvsb", bufs=4))
    rc_pool = ctx.enter_context(tc.tile_pool(name="rc", bufs=4))
    rb_pool = ctx.enter_context(tc.tile_pool(name="rb", bufs=2))
    # PSUM budget (8 banks): scores 2x[128,1024]f32 = 4, pv 1x[65,1024]f32 = 2,
    # proj/transpose/outproj 2x[128,512] = 2.
    ps_s = ctx.enter_context(tc.tile_pool(name="ps_s", bufs=2, space="PSUM"))
    ps_pv = ctx.enter_context(tc.tile_pool(name="ps_pv", bufs=1, space="PSUM"))
    ps_w = ctx.enter_context(tc.tile_pool(name="ps_w", bufs=2, space="PSUM"))
    dram_pool = ctx.enter_context(tc.tile_pool(name="dram", bufs=4, space="DRAM"))

    # ---- constants ----
    # gpsimd/memset can't emit float32r, so build fp32 then copy-round on DVE
    # (0.0/1.0 are exactly representable, so the copy is exact).
    ident_f32 = const.tile([P, P], F32, tag="ident_f32")
    from concourse.masks import make_identity
    make_identity(nc, ident_f32[:])
    identity = const.tile([P, P], BF16, tag="ident")
    nc.vector.tensor_copy(identity[:], ident_f32[:])
    bq_sb = const.tile([P, KT], F32, tag="bq")
    nc.sync.dma_start(bq_sb[:], bq_d.ap().rearrange("(t p) -> p t", p=P))
    bk_sb = const.tile([P, KT], F32, tag="bk")
    nc.sync.dma_start(bk_sb[:], bk_d.ap().rearrange("(t p) -> p t", p=P))
    # bo broadcast to all 128 partitions via a 0-step partition AP (DRAM APs
    # are not partitioned, so a 0-step leading dim is legal here)
    bo_bc = const.tile([P, E], F32, tag="bo")
    bo_ap = bo_d.ap()
    bo_bcast_src = bass.AP(bo_ap.tensor, bo_ap.offset, [[0, P], [1, E]])
    nc.sync.dma_start(bo_bc[:], bo_bcast_src)

    # ---- input loads (per k-tile so compute can start early) ----
    xT_sb = xt_pool.tile([P, KT, S], BF16, tag="xt")
    WqT_sb = w_pool.tile([P, KT, E], BF16, tag="w")
    WkT_sb = w_pool.tile([P, KT, E], BF16, tag="w")
    WoT_sb = wo_pool.tile([P, KT, E], BF16, tag="wo")
    xT_r = xT_d.ap().rearrange("(t p) s -> p t s", p=P)
    WqT_r = WqT_d.ap().rearrange("(t p) e -> p t e", p=P)
    WkT_r = WkT_d.ap().rearrange("(t p) e -> p t e", p=P)
    WoT_r = WoT_d.ap().rearrange("(t p) e -> p t e", p=P)
    for t in range(KT):
        nc.sync.dma_start(xT_sb[:, t, :], xT_r[:, t, :])
        nc.sync.dma_start(WqT_sb[:, t, :], WqT_r[:, t, :])
        nc.sync.dma_start(WkT_sb[:, t, :], WkT_r[:, t, :])
        nc.sync.dma_start(WoT_sb[:, t, :], WoT_r[:, t, :])

    vaug = vaug_pool.tile([P, ST, H, DH + 1], BF16, tag="vaug")
    for jb in range(ST):
        nc.vector.memset(vaug[:, jb, :, DH:DH + 1], 1.0)
    outT_sb = outt_pool.tile([P, KT, S], BF16, tag="outt")

    # warm the exp activation-table set once, outside the hardware loop, so
    # ACT_TABLE_LOAD (~1.3us) isn't paid per iteration
    wsrc = const.tile([1, 4], F32, tag="wsrc")
    nc.vector.memset(wsrc[:], 0.0)
    wdst = const.tile([1, 4], F32, tag="wdst")
    nc.scalar.activation(wdst[:], wsrc[:], Exp, scale=1.0)

    # ---- pair preparation: projections + vaug transposes for head pair hp,
    # written as a generator that yields after every couple of PE ops. The
    # attention loop of pair hp-1 drains it a few steps per j-block, so this
    # PE work fills the idle slivers while ACT runs exp — keeping the tensor
    # engine dense enough that the HAM clock gate stays at full speed.
    pair_qk = {}

    def pair_prep(hp, qk=None):
        if qk is None:
            qp = qt_pool.tile([P, S], BF16, tag="qt", name=f"qp_{hp}")
            kp = kt_pool.tile([P, S], BF16, tag="kt", name=f"kp_{hp}")
        else:
            qp, kp = qk
        pair_qk[hp] = (qp, kp)
        for W_sb, b_sb, out_sb in ((WqT_sb, bq_sb, qp), (WkT_sb, bk_sb, kp)):
            for c in range(NCH):
                ps = ps_w.tile([P, 512], F32, tag="ps_w")
                for t in range(KT):
                    nc.tensor.matmul(
                        ps[:],
                        W_sb[:, t, 128 * hp:128 * hp + 128],
                        xT_sb[:, t, 512 * c:512 * c + 512],
                        start=(t == 0), stop=(t == KT - 1),
                    )
                    if t % 2 == 1:
                        yield
                nc.vector.tensor_scalar_add(
                    out_sb[:, 512 * c:512 * c + 512], ps[:], b_sb[:, hp:hp + 1]
                )
                yield
        # vaug slices for heads 2hp, 2hp+1 via PE transposes of kT tile hp
        for g in range(2):
            ps = ps_w.tile([P, 512], BF16, tag="ps_w")
            for j4 in range(4):
                jb = 4 * g + j4
                nc.tensor.transpose(
                    ps[:, 128 * j4:128 * j4 + 128],
                    kp[:, 128 * jb:128 * jb + 128],
                    identity[:],
                )
                if j4 % 2 == 1:
                    yield
            nc.vector.tensor_copy(
                vaug[:, 4 * g:4 * g + 4, 2 * hp:2 * hp + 2, 0:DH],
                ps[:].rearrange("p (a b c) -> p a b c", a=4, b=2, c=DH),
            )
            yield

    def drain(gen, n):
        if gen is None:
            return None
        for _ in range(n):
            try:
                next(gen)
            except StopIteration:
                return None
        return gen

    # partial output projection: the t=0..3 outT tiles only depend on head
    # pairs 0..3, so their contributions to y accumulate during pair 5's
    # attention (as its interleave filler) and evict early into ysb partials;
    # only the t=4,5 matmuls remain for the epilogue after the last head.
    ysb_tiles = {}

    def outproj_prep():
        for st in range(ST):
            ysb = ysb_pool.tile([P, E], F32, tag="ysb", name=f"ysb_{st}")
            ysb_tiles[st] = ysb
            for n0 in (0, 384):
                ps = ps_w.tile([P, 512], F32, tag="ps_w")
                for t in range(KT - 2):
                    nc.tensor.matmul(
                        ps[:, 0:384],
                        outT_sb[:, t, 128 * st:128 * st + 128],
                        WoT_sb[:, t, n0:n0 + 384],
                        start=(t == 0), stop=(t == KT - 3),
                    )
                    if t % 2 == 1:
                        yield
                nc.vector.tensor_add(
                    ysb[:, n0:n0 + 384], ps[:, 0:384], bo_bc[:, n0:n0 + 384])
                yield

    # prologue: pair 0 prepared densely before its attention starts. Inside
    # the hardware loop the NEXT iteration's pair-0 prep is re-emitted into
    # the same tiles at the tail (chained after the outproj partials), so it
    # hides inside the last head's normalize latency instead of stalling the
    # next iteration's start.
    import itertools
    for _ in pair_prep(0):
        pass
    qk0 = pair_qk[0]

    if iters > 1:
        ctx.enter_context(tc.For_i(0, iters, 1))

    for hp in range(KT):
        qp, kp = pair_qk.pop(hp)
        last_pair = hp + 1 >= KT
        if last_pair:
            gen = outproj_prep()
            if iters > 1:
                gen = itertools.chain(gen, pair_prep(0, qk=qk0))
        else:
            gen = pair_prep(hp + 1)
        nsteps = 3 if last_pair else 2
        for h in (2 * hp, 2 * hp + 1):
            po = DH * (h % 2)
            pv = ps_pv.tile([DH + 1, S], F32, tag="ps_pv", name=f"pv_{h}")

            def pv_mms(jb, pt):
                for c in range(NCH):
                    nc.tensor.matmul(
                        pv[:, 512 * c:512 * c + 512],
                        vaug[:, jb, h, :],
                        pt[:, 512 * c:512 * c + 512],
                        start=(jb == 0), stop=(jb == ST - 1),
                    )

            # software-pipelined by one j-block: the PE issues scores(jb)
            # before PV(jb-1), so exp(jb-1) on ACT overlaps scores(jb) on PE
            # instead of stalling the PE.
            prev = None
            for jb in range(ST):
                sps = ps_s.tile([P, S], F32, tag="ps_s", name=f"sps_{h}_{jb}")
                for c in range(NCH):
                    nc.tensor.matmul(
                        sps[:, 512 * c:512 * c + 512],
                        kp[po:po + DH, 128 * jb:128 * jb + 128],
                        qp[po:po + DH, 512 * c:512 * c + 512],
                        start=True, stop=True,
                    )
                pt = pt_pool.tile([P, S], BF16, tag="pt")
                nc.scalar.activation(pt[:], sps[:], Exp, scale=SCALE)
                gen = drain(gen, nsteps)
                if prev is not None:
                    pv_mms(jb - 1, prev)
                prev = pt
            pv_mms(ST - 1, prev)
            # evict pv to SBUF right away (frees the PSUM bank), then
            # normalize: reciprocal of the rowsum row, broadcast across 64
            # partitions via a DRAM round-trip (DRAM APs allow a 0-step
            # partition dim), multiply into outT. Keeps the PE entirely out
            # of the normalization chain.
            pvsb = pvsb_pool.tile([DH + 1, S], F32, tag="pvsb", name=f"pvsb_{h}")
            nc.vector.tensor_copy(pvsb[:], pv[:])
            # reciprocal of the rowsum (pv row DH): a [1, S] DVE reciprocal
            # is ~6.4ns/elem on a single lane (6.5us), so spread it across a
            # [128, S/128] layout with an SBUF->SBUF DMA (dst walks p-major:
            # rs8[p, e] = rowsum[8p + e]), reciprocal on all 128 lanes
            # (~0.2us), scatter back to a partition-0 [1, S] tile, and
            # broadcast across 64 partitions on the idle GPSIMD engine.
            # (DVE operand base partitions must stay 32-aligned: inputs to
            # the mul are all at base 0.)
            rs8 = rc_pool.tile([P, S // P], F32, tag="rs8", name=f"rs8_{h}")
            nc.sync.dma_start(rs8[:], pvsb[DH:DH + 1, :])
            rc8 = rc_pool.tile([P, S // P], F32, tag="rc8", name=f"rc8_{h}")
            nc.vector.reciprocal(rc8[:], rs8[:])
            rcl = rc_pool.tile([1, S], F32, tag="rcl", name=f"rcl_{h}")
            nc.sync.dma_start(rcl[:], rc8[:])
            rb = rb_pool.tile([DH, S], F32, tag="rb", name=f"rb_{h}")
            nc.gpsimd.partition_broadcast(rb[:], rcl[:])
            nc.vector.tensor_mul(
                outT_sb[po:po + DH, hp, :], pvsb[0:DH, :], rb[:],
            )

        drain(gen, 999)

    # ---- output projection epilogue: the last two outT tiles (pairs 4, 5)
    # into each ysb partial, then store y ----
    y_r = y_d.ap().rearrange("(st p) e -> st p e", p=P)
    for st in range(ST):
        ysb = ysb_tiles.pop(st)
        for n0 in (0, 384):
            yps = ps_w.tile([P, 512], F32, tag="ps_w")
            for t in (KT - 2, KT - 1):
                nc.tensor.matmul(
                    yps[:, 0:384],
                    outT_sb[:, t, 128 * st:128 * st + 128],
                    WoT_sb[:, t, n0:n0 + 384],
                    start=(t == KT - 2), stop=(t == KT - 1),
                )
            nc.vector.tensor_add(ysb[:, n0:n0 + 384], ysb[:, n0:n0 + 384], yps[:, 0:384])
        nc.sync.dma_start(y_r[st], ysb[:])


_NC_CACHE = {}


def build(iters=1, variant="full"):
    key = (iters, variant)
    nc = _NC_CACHE.get(key)
    if nc is None:
        nc = bacc.Bacc("TRN2", target_bir_lowering=False, debug=False)
        with tile.TileContext(nc) as tc, ExitStack() as ctx:
            _emit(nc, tc, ctx, iters=iters, variant=variant)
        nc.compile()
        _NC_CACHE[key] = nc
    return nc


def _to_bf16(a):
    return np.ascontiguousarray(
        np.asarray(a, dtype=np.float32)).astype(mybir.dt.np(mybir.dt.bfloat16))


def make_in_maps(x, Wq, bq, Wk, bk, Wo, bo):
    WqT = _to_bf16(np.asarray(Wq, dtype=np.float32).T)
    WkT = _to_bf16(np.asarray(Wk, dtype=np.float32).T)
    WoT = _to_bf16(np.asarray(Wo, dtype=np.float32).T)
    bq = np.ascontiguousarray(np.asarray(bq, dtype=np.float32))
    bk = np.ascontiguousarray(np.asarray(bk, dtype=np.float32))
    bo = np.ascontiguousarray(np.asarray(bo, dtype=np.float32))
    x = np.asarray(x, dtype=np.float32)
    return [
        {
            "xT": _to_bf16(x[c].T),
            "WqT": WqT, "WkT": WkT, "WoT": WoT,
            "bq": bq, "bk": bk, "bo": bo,
        }
        for c in range(NCORES)
    ]


def kernel(x, Wq, bq, Wk, bk, Wo, bo):
    nc = build()
    in_maps = make_in_maps(x, Wq, bq, Wk, bk, Wo, bo)
    res = bass_utils.run_bass_kernel_spmd(nc, in_maps, core_ids=list(range(NCORES)))
    return np.stack([res.results[c]["y"] for c in range(NCORES)]).astype(np.float32)

